# revision 30
# baseline (speedup 1.0000x reference)
"""Attention2d Trainium2 kernel.

Sharding: 16 heads / 8 cores = 2 heads per core, data-parallel over all 4
batches on every core (head sharding minimizes rel_pos traffic: each core
reads only its 2 heads' [N, N] slices). The output projection contracts over
all heads' channels, so each core emits a partial projection output over its
64 channels; the host sums the 8 partials and adds b_proj.

Device pipeline per (batch, head) pair:
  qkv     = wqkv^T @ x_b           (PE; x and wqkv in fp8 with a x16 weight
                                    scale folded host-side; ACT bias-evict)
  v^T     single DMA-transpose [32,1024] -> [128,8,32] (no engine time)
  q, k    replicated across 32-partition row groups by SBUF->SBUF DMA so the
          K=32 score matmuls pack 4-wide via tile_position row tiling
  scores  4 rounds; round r = chunks jc=r (c0) and jc=r+4 (c1), each a
          [128, 1024] half-round -> one 4-bank psum tile; the softmax reads
          are split per half-round so the next round's matmuls only wait on
          their own bank pair (WAR at half-round granularity)
  p       per chunk, one of: ACT exp(scale=1/256) then *exp(R^T) on DVE
          (fp16 2x) or GpSimd (2 chunks, consumed 2 rounds later); or a
          fused Schraudolph exp-approx on DVE (scalar_tensor_tensor:
          int16(S*c1/256 + R''), bitcast fp16; R'' host-precomputed) for
          chunks jc in {3, 6, 7} - removes both the ACT op and the multiply
  att     [vt|1]^T @ p accumulated across rounds at lag 2 (PE 128x128 mode;
          i halves in PE output quadrants 0/64; ones column makes rows
          32/96 the denominators)
  rcp     denominator rows -> DVE copy -> PE ones-broadcast -> DVE recip
  att_sb  = att * rcp on DVE; h0 lands directly in the proj rhs tile, h1 is
          DMA-shifted so rows hold [h0|h1] x [i-half0|i-half1]
  out_b  += wp64^T @ att_sb        (PE 64x128 mode, K=64 contracts both
                                    heads, 2 concurrent row tiles; one
                                    2-bank eviction per output chunk)
"""

import sys

sys.path.insert(0, "/opt/trn_rl_repo")

import numpy as np
import ml_dtypes

import concourse.bass as bass
import concourse.tile as tile
from concourse import mybir, bacc
from concourse.bass_utils import run_bass_kernel_spmd

B, C = 4, 512
N = 1024  # 32*32 pixels
HEADS, DH = 16, 32
NCORES = 8
HPC = HEADS // NCORES  # heads per core
F16 = mybir.dt.float16
F32 = mybir.dt.float32
F8 = mybir.dt.float8e4
I16 = mybir.dt.int16
AF = mybir.ActivationFunctionType
OP = mybir.AluOpType

WSCALE = 16.0  # fp8 weight scale (q,k,v each x16 -> scores x256)
SEXP = 1.0 / (WSCALE * WSCALE)
C1 = 1477.3197  # 2^10 / ln(2)
C2 = 15315.27  # 15 * 1024 - 44.73 (L-inf centered Schraudolph)
SCH_JC = (0, 1, 4)  # chunks using the fused DVE exp-approx (early rounds so
# the pair boundary never waits on a deep DVE queue)
GPS_JC = (2, 5)  # chunks whose rel-pos multiply runs on GpSimd

_BUILT = None


def build_nc():
    nc = bacc.Bacc("TRN2", target_bir_lowering=False, debug=False, num_devices=NCORES)
    x8 = nc.declare_dram_parameter("x8", [B, C, N], F8, isOutput=False)
    wqkvT = nc.declare_dram_parameter("wqkvT", [C, HPC, 96], F8, isOutput=False)
    bqkv = nc.declare_dram_parameter("bqkv", [96, HPC], F32, isOutput=False)
    wp64T = nc.declare_dram_parameter("wp64T", [128, 4, 128], F16, isOutput=False)
    rt = nc.declare_dram_parameter("rt", [HPC, N, N], F16, isOutput=False)
    outp = nc.declare_dram_parameter("outp", [B, C, N], F16, isOutput=True)

    with tile.TileContext(nc) as tc:
        with (
            tc.tile_pool(name="singles", bufs=1) as singles,
            tc.tile_pool(name="work", bufs=2) as work,
            tc.tile_pool(name="ps", bufs=1, space="PSUM") as pspool,
        ):
            # ---- preamble: constants + resident tensors ----
            ones_bc = singles.tile([128, 32], F16)
            nc.vector.memset(ones_bc[:], 1.0)

            # DoubleRow channel pairing: c = cc*256 + ki*2 + ko on both the
            # weight and activation side (any consistent bijection works)
            wq_sb = singles.tile([128, 2, 2, HPC, 96], F8)
            nc.sync.dma_start(
                wq_sb[:], wqkvT.rearrange("(cc p ko) h m -> p cc ko h m", p=128, ko=2)
            )
            bq_sb = singles.tile([96, HPC], F32)
            nc.sync.dma_start(bq_sb[:], bqkv[:])
            wp_sb = singles.tile([128, 4, 128], F16)
            nc.sync.dma_start(wp_sb[:], wp64T[:])

            # input DMAs ordered by first use so pair 0 can start early
            xb_sb = singles.tile([128, B, 2, 2, N], F8)
            expRT = [
                singles.tile(
                    [128, 2, 4, 2, 512], F16, tag=f"expRT{h}", name=f"expRT{h}"
                )
                for h in range(HPC)
            ]
            x8r = x8.rearrange("b (cc p ko) n -> b p cc ko n", p=128, ko=2)
            rtr = rt.rearrange("h (jc p) (u n) -> h p jc u n", p=128, u=2)

            def load_rt(h, eng):
                for r in range(4):
                    for c in range(2):
                        eng.dma_start(expRT[h][:, c, r], rtr[h, :, 4 * c + r])

            # interleave by first use across both hwdge trigger queues:
            # sync feeds pair 0 (x b0 + rt h0), scalar feeds the rest
            for cc in range(2):
                nc.sync.dma_start(xb_sb[:, 0, cc], x8r[0, :, cc])
            load_rt(1, nc.scalar)
            nc.scalar.dma_start(xb_sb[:, 1], x8r[1])
            load_rt(0, nc.sync)
            for b in range(2, B):
                nc.scalar.dma_start(xb_sb[:, b], x8r[b])

            # ---- per-pair stage emitters ----
            def emit_qkv_mm(b, h, idx, nn):
                ps_qkv = pspool.tile(
                    [128, 2, 4, 128], F32, tag="big", bufs=1, name="ps_qkv"
                )
                for cc in range(2):
                    nc.tensor.matmul(
                        ps_qkv[0:96, nn],
                        lhsT=wq_sb[:, cc, :, h, :],
                        rhs=xb_sb[:, b, cc, :, 512 * nn : 512 * nn + 512],
                        start=(cc == 0),
                        stop=(cc == 1),
                        perf_mode=mybir.MatmulPerfMode.DoubleRow,
                    )
                return ps_qkv

            def emit_qkv_evict(h, ps_qkv, qkv_hold, nn):
                # bias-add eviction on ACT (keeps DVE free; bias is a
                # per-partition scalar so ACT can fuse it)
                nc.scalar.activation(
                    qkv_hold[:, 4 * nn : 4 * nn + 4, :],
                    ps_qkv[0:96, nn],
                    AF.Identity,
                    bias=bq_sb[:, h : h + 1],
                )

            def emit_repl(qkv_hold):
                # replicate q to row groups 1-3; k chunk-blocks to groups
                # 0-1 (jc 0-3) and 2-3 (jc 4-7); transpose v in one DMA
                q4 = work.tile([128, 8, 128], F16, tag="q4")
                kk = work.tile([128, 4, 128], F16, tag="kk")
                vtc = work.tile([128, 8, 32], F16, tag="vtc")
                vt1 = work.tile([128, 8, 34], F16, tag="vt1")
                for g in range(1, 4):
                    nc.sync.dma_start(q4[32 * g : 32 * g + 32], qkv_hold[0:32])
                for g in range(4):
                    nc.gpsimd.dma_start(
                        kk[32 * g : 32 * g + 32],
                        qkv_hold[32:64, 4 * (g // 2) : 4 * (g // 2) + 4, :],
                    )
                # xbar transpose needs a fully contiguous destination; the
                # strided [.., 0:32] view of vt1 is re-packed by a cheap 4x
                # DVE copy that also leaves room for the ones column
                nc.sync.dma_start_transpose(vtc[:], qkv_hold[64:96])
                nc.vector.tensor_copy(vt1[:, :, 0:32], vtc[:])
                nc.vector.memset(vt1[:, :, 32:33], 1.0)
                return q4, kk, vt1

            def emit_mid(b, h, qkv_hold, q4, kk, vt1, deferred, fin_late, inject):
                p2 = work.tile([128, 2, 4, 2, 512], F16, tag="p2")
                att_ps = pspool.tile([128, 512], F32, tag="att", bufs=2)

                def attv(r):
                    # the two i-half quadrants run concurrently as column
                    # tiles (0,0)/(0,64) - each M=33 fits a 64-col tile
                    for cg in range(4):
                        c, nn = cg // 2, cg % 2
                        jc = 4 * c + r
                        nc.tensor.matmul(
                            att_ps[64 * nn : 64 * nn + 33, :],
                            lhsT=vt1[:, jc, 0:33],
                            rhs=p2[:, c, r, nn, :],
                            start=(jc == 0),
                            stop=(jc == 7),
                            tile_position=(0, 64 * nn),
                        )

                def chunk(sc_ps, r, c):
                    jc = 4 * c + r
                    if jc in SCH_JC:
                        # fused exp-approx: int16(S*C1/256 + R'') bits = fp16
                        nc.vector.scalar_tensor_tensor(
                            out=p2[:, c, r].bitcast(I16),
                            in0=sc_ps[:],
                            scalar=C1 * SEXP,
                            in1=expRT[h][:, c, r],
                            op0=OP.mult,
                            op1=OP.add,
                        )
                    else:
                        nc.scalar.activation(
                            p2[:, c, r], sc_ps[:], AF.Exp, scale=SEXP
                        )
                        eng = nc.gpsimd if jc in GPS_JC else nc.vector
                        eng.tensor_tensor(
                            p2[:, c, r], p2[:, c, r], expRT[h][:, c, r], OP.mult
                        )

                for r in range(4):
                    for c in range(2):
                        # half-round: one jc chunk, both i-halves, on its own
                        # 2-bank psum (bufs=2 gives 2 half-rounds of slack so
                        # score matmuls never wait on the softmax consumers)
                        sc_ps = pspool.tile(
                            [128, 2, 512], F32, tag="sc", bufs=2, name="sc_ps"
                        )
                        for nn in range(2):
                            g = 2 * c + nn
                            nc.tensor.matmul(
                                sc_ps[:, nn, :],
                                lhsT=kk[32 * g : 32 * g + 32, r, :],
                                rhs=(qkv_hold if g == 0 else q4)[
                                    32 * g : 32 * g + 32, 4 * nn : 4 * nn + 4, :
                                ],
                                start=True,
                                stop=True,
                                tile_position=(32 * g, 0),
                            )
                        chunk(sc_ps, r, c)
                    if r >= 2:
                        attv(r - 2)
                    if r in (0, 1) and inject:
                        inject.pop(0)()
                    if r >= 2 and deferred:
                        deferred.pop(0)()
                attv(2)
                attv(3)
                if deferred:
                    deferred.pop(0)()
                return att_ps

            def emit_fin_early(att_ps):
                # denominators: rows 32/96 of att_ps -> sbuf (DVE)
                cs = work.tile([128, 512], F16, tag="cs")
                for nn in range(2):
                    rr = 64 * nn + 32
                    nc.vector.tensor_copy(cs[rr : rr + 1, :], att_ps[rr : rr + 1, :])
                return cs

            def emit_fin_late(att_ps, cs, dst, dst_half):
                bc_ps = pspool.tile([128, 512], F32, tag="att", bufs=2, name="bc_ps")
                for nn in range(2):
                    rr = 64 * nn + 32
                    nc.tensor.matmul(
                        bc_ps[64 * nn : 64 * nn + 32, :],
                        lhsT=ones_bc[rr : rr + 1, 0:32],
                        rhs=cs[rr : rr + 1, :],
                        start=True,
                        stop=True,
                        tile_position=(rr, 64 * nn),
                    )
                rcp = work.tile([128, 512], F32, tag="rcp")
                nc.vector.reciprocal_approx_fast(rcp[:], bc_ps[:])
                nc.vector.tensor_tensor(dst[:], att_ps[:], rcp[:], OP.mult)
                if dst_half is not None:
                    # h1: shift quadrants down 32 partitions into the proj rhs
                    nc.sync.dma_start(dst_half[32:64], dst[0:32])
                    nc.sync.dma_start(dst_half[96:128], dst[64:96])

            def make_proj(b, proj_rhs):
                # 4 deferred chunks; each: two concurrent 64x128 row-tiled
                # matmuls (K=64 contracts both heads), one 2-bank eviction,
                # output DMA
                out_sb = work.tile([128, 4, 2, 512], F16, tag="out_sb")
                outr = outp[b].rearrange("(oc p) (u n) -> p oc u n", p=128, u=2)

                def chunk(oc):
                    def run():
                        # the last batch's chunks run in the drain when the
                        # score psum is idle - rotate through its buffers so
                        # consecutive chunks never wait on each other's evict
                        if b == B - 1:
                            pj = pspool.tile(
                                [128, 2, 512], F32, tag="sc", bufs=2, name="sc_ps"
                            )
                        else:
                            pj = pspool.tile(
                                [128, 2, 512], F32, tag="big", bufs=1, name="pj"
                            )
                        for nn in range(2):
                            nc.tensor.matmul(
                                pj[:, nn, :],
                                lhsT=wp_sb[64 * nn : 64 * nn + 64, oc, :],
                                rhs=proj_rhs[64 * nn : 64 * nn + 64, :],
                                start=True,
                                stop=True,
                                tile_position=(64 * nn, 0),
                            )
                        if oc % 2 == 0:
                            nc.scalar.activation(out_sb[:, oc], pj[:], AF.Identity)
                        else:
                            nc.vector.tensor_copy(out_sb[:, oc], pj[:])
                        nc.sync.dma_start(outr[:, oc], out_sb[:, oc])

                    return run

                return [chunk(oc) for oc in range(4)]

            # ---- main loop, software-pipelined across pairs ----
            pairs = [(b, h) for b in range(B) for h in range(HPC)]
            qkv_tiles = {}
            proj_tiles = {}
            deferred = []

            def make_qkv_inject(idx):
                b, h = pairs[idx]

                def half0():
                    qkv_hold = work.tile([96, 8, 128], F16, tag="qkv_hold")
                    ps_qkv = emit_qkv_mm(b, h, idx, 0)
                    emit_qkv_evict(h, ps_qkv, qkv_hold, 0)
                    qkv_tiles[idx] = [qkv_hold, ps_qkv, None, None, None]

                def half1():
                    st = qkv_tiles[idx]
                    qkv_hold, ps_qkv = st[0], st[1]
                    for cc in range(2):
                        nc.tensor.matmul(
                            ps_qkv[0:96, 1],
                            lhsT=wq_sb[:, cc, :, h, :],
                            rhs=xb_sb[:, b, cc, :, 512:1024],
                            start=(cc == 0),
                            stop=(cc == 1),
                            perf_mode=mybir.MatmulPerfMode.DoubleRow,
                        )
                    emit_qkv_evict(h, ps_qkv, qkv_hold, 1)
                    st[2], st[3], st[4] = emit_repl(qkv_hold)

                return [half0, half1]

            def make_fin_late(pb, ph, patt, cs):
                def run():
                    if ph == 0:
                        proj_rhs = work.tile([128, 512], F16, tag="proj_rhs")
                        proj_tiles[pb] = proj_rhs
                        emit_fin_late(patt, cs, proj_rhs, None)
                    else:
                        proj_rhs = proj_tiles.pop(pb)
                        att_h1 = work.tile([128, 512], F16, tag="att_h1")
                        emit_fin_late(patt, cs, att_h1, proj_rhs)
                        deferred.extend(make_proj(pb, proj_rhs))

                return run

            for fn in make_qkv_inject(0):
                fn()
            for idx, (b, h) in enumerate(pairs):
                qkv_hold, _ps, q4, kk, vt1 = qkv_tiles.pop(idx)
                inject = make_qkv_inject(idx + 1) if idx + 1 < len(pairs) else []
                att_ps = emit_mid(
                    b, h, qkv_hold, q4, kk, vt1, deferred, None, inject
                )
                # fin chain inline at the pair tail: the DVE/PE ops land
                # ahead of the next pair's queue work, so proj and the next
                # pair's scores never wait on a deep queue
                cs = emit_fin_early(att_ps)
                make_fin_late(b, h, att_ps, cs)()
            for fn in deferred:
                fn()

    nc.compile()
    return nc


def _get_nc():
    global _BUILT
    if _BUILT is None:
        _BUILT = build_nc()
    return _BUILT


def _prep_inputs(x, w_qkv, b_qkv, w_proj, b_proj, shared_rel_pos):
    """Host-side sharding/layout prep. Returns per-core input maps."""
    scale = np.float32(DH**-0.5)
    x8 = np.ascontiguousarray(x.reshape(B, C, N)).astype(ml_dtypes.float8_e4m3)

    wq = w_qkv.reshape(HEADS, 96, C).astype(np.float32) * WSCALE
    wq[:, 0:32, :] *= scale  # fold attention scale into q
    bq = b_qkv.reshape(HEADS, 96).astype(np.float32) * WSCALE
    bq[:, 0:32] *= scale

    in_maps = []
    for g in range(NCORES):
        hh = [HPC * g + h for h in range(HPC)]
        wqkvT = np.ascontiguousarray(
            wq[hh].transpose(2, 0, 1).astype(ml_dtypes.float8_e4m3)
        )  # [C, HPC, 96]
        bqkv = np.ascontiguousarray(bq[hh].T)  # [96, HPC]
        # proj weight columns (un-scaled by 1/WSCALE): [64 (h d), 4 oc, 128]
        # stacked twice along partitions for the two 64x128 row tiles
        wp = w_proj[:, 64 * g : 64 * (g + 1)].astype(np.float32) / WSCALE
        wp64 = wp.T.reshape(64, 4, 128)
        wp64T = np.ascontiguousarray(
            np.concatenate([wp64, wp64], axis=0).astype(np.float16)
        )
        # rel-pos: exp(R^T) for ACT-exp chunks, Schraudolph R'' for SCH_JC
        rT = shared_rel_pos[0, hh].transpose(0, 2, 1).astype(np.float32)  # [h, j, i]
        rt = np.exp(rT)
        for jc in SCH_JC:
            sl = slice(128 * jc, 128 * (jc + 1))
            rt[:, sl, :] = rT[:, sl, :] * C1 + C2
        rt = np.ascontiguousarray(rt).astype(np.float16)
        in_maps.append(
            {"x8": x8, "wqkvT": wqkvT, "bqkv": bqkv, "wp64T": wp64T, "rt": rt}
        )
    return in_maps


def kernel(x, w_qkv, b_qkv, w_proj, b_proj, shared_rel_pos, _trace=False):
    nc = _get_nc()
    in_maps = _prep_inputs(x, w_qkv, b_qkv, w_proj, b_proj, shared_rel_pos)
    res = run_bass_kernel_spmd(nc, in_maps, list(range(NCORES)), trace=_trace)
    kernel.last_result = res
    out = np.zeros((B, C, N), np.float32)
    for g in range(NCORES):
        out += res.results[g]["outp"].astype(np.float32)
    out += b_proj.astype(np.float32)[None, :, None]
    return out.reshape(B, C, 32, 32).astype(np.float32)


# revision 32
# speedup vs baseline: 1.0175x; 1.0175x over previous
"""Attention2d Trainium2 kernel.

Sharding: 16 heads / 8 cores = 2 heads per core, data-parallel over all 4
batches on every core (head sharding minimizes rel_pos traffic: each core
reads only its 2 heads' [N, N] slices). The output projection contracts over
all heads' channels, so each core emits a partial projection output over its
64 channels; the host sums the 8 partials and adds b_proj.

Device pipeline per (batch, head) pair:
  qkv     = wqkv^T @ x_b           (PE; x and wqkv in fp8 with a x16 weight
                                    scale folded host-side; ACT bias-evict)
  v^T     single DMA-transpose [32,1024] -> [128,8,32] (no engine time)
  q, k    replicated across 32-partition row groups by SBUF->SBUF DMA so the
          K=32 score matmuls pack 4-wide via tile_position row tiling
  scores  4 rounds; round r = chunks jc=r (c0) and jc=r+4 (c1), each a
          [128, 1024] half-round -> one 4-bank psum tile; the softmax reads
          are split per half-round so the next round's matmuls only wait on
          their own bank pair (WAR at half-round granularity)
  p       per chunk, one of: ACT exp(scale=1/256) then *exp(R^T) on DVE
          (fp16 2x) or GpSimd (2 chunks, consumed 2 rounds later); or a
          fused Schraudolph exp-approx on DVE (scalar_tensor_tensor:
          int16(S*c1/256 + R''), bitcast fp16; R'' host-precomputed) for
          chunks jc in {3, 6, 7} - removes both the ACT op and the multiply
  att     [vt|1]^T @ p accumulated across rounds at lag 2 (PE 128x128 mode;
          i halves in PE output quadrants 0/64; ones column makes rows
          32/96 the denominators)
  rcp     denominator rows -> DVE copy -> PE ones-broadcast -> DVE recip
  att_sb  = att * rcp on DVE; h0 lands directly in the proj rhs tile, h1 is
          DMA-shifted so rows hold [h0|h1] x [i-half0|i-half1]
  out_b  += wp64^T @ att_sb        (PE 64x128 mode, K=64 contracts both
                                    heads, 2 concurrent row tiles; one
                                    2-bank eviction per output chunk)
"""

import sys

sys.path.insert(0, "/opt/trn_rl_repo")

import numpy as np
import ml_dtypes

import concourse.bass as bass
import concourse.tile as tile
from concourse import mybir, bacc
from concourse.bass_utils import run_bass_kernel_spmd

B, C = 4, 512
N = 1024  # 32*32 pixels
HEADS, DH = 16, 32
NCORES = 8
HPC = HEADS // NCORES  # heads per core
F16 = mybir.dt.float16
F32 = mybir.dt.float32
F8 = mybir.dt.float8e4
I16 = mybir.dt.int16
AF = mybir.ActivationFunctionType
OP = mybir.AluOpType

WSCALE = 16.0  # fp8 weight scale (q,k,v each x16 -> scores x256)
SEXP = 1.0 / (WSCALE * WSCALE)
C1 = 1477.3197  # 2^10 / ln(2)
C2 = 15315.27  # 15 * 1024 - 44.73 (L-inf centered Schraudolph)
SCH_JC = (0, 1, 4)  # chunks using the fused DVE exp-approx (early rounds so
# the pair boundary never waits on a deep DVE queue)
GPS_JC = (2, 5)  # chunks whose rel-pos multiply runs on GpSimd

_BUILT = None


def build_nc():
    nc = bacc.Bacc("TRN2", target_bir_lowering=False, debug=False, num_devices=NCORES)
    x8 = nc.declare_dram_parameter("x8", [B, C, N], F8, isOutput=False)
    wqkvT = nc.declare_dram_parameter("wqkvT", [C, HPC, 96], F8, isOutput=False)
    bqkv = nc.declare_dram_parameter("bqkv", [96, HPC], F32, isOutput=False)
    wp64T = nc.declare_dram_parameter("wp64T", [128, 4, 128], F16, isOutput=False)
    rt = nc.declare_dram_parameter("rt", [HPC, N, N], F16, isOutput=False)
    outp = nc.declare_dram_parameter("outp", [B, C, N], F16, isOutput=True)

    with tile.TileContext(nc) as tc:
        with (
            tc.tile_pool(name="singles", bufs=1) as singles,
            tc.tile_pool(name="work", bufs=2) as work,
            tc.tile_pool(name="ps", bufs=1, space="PSUM") as pspool,
        ):
            # ---- preamble: constants + resident tensors ----
            ones_bc = singles.tile([128, 32], F16)
            nc.vector.memset(ones_bc[:], 1.0)

            # DoubleRow channel pairing: c = cc*256 + ki*2 + ko on both the
            # weight and activation side (any consistent bijection works)
            wq_sb = singles.tile([128, 2, 2, HPC, 96], F8)
            nc.sync.dma_start(
                wq_sb[:], wqkvT.rearrange("(cc p ko) h m -> p cc ko h m", p=128, ko=2)
            )
            bq_sb = singles.tile([96, HPC], F32)
            nc.sync.dma_start(bq_sb[:], bqkv[:])
            wp_sb = singles.tile([128, 4, 128], F16)
            nc.sync.dma_start(wp_sb[:], wp64T[:])

            # input DMAs ordered by first use so pair 0 can start early
            xb_sb = singles.tile([128, B, 2, 2, N], F8)
            expRT = [
                singles.tile(
                    [128, 2, 4, 2, 512], F16, tag=f"expRT{h}", name=f"expRT{h}"
                )
                for h in range(HPC)
            ]
            x8r = x8.rearrange("b (cc p ko) n -> b p cc ko n", p=128, ko=2)
            rtr = rt.rearrange("h (jc p) (u n) -> h p jc u n", p=128, u=2)

            def load_rt(h, eng):
                for r in range(4):
                    for c in range(2):
                        eng.dma_start(expRT[h][:, c, r], rtr[h, :, 4 * c + r])

            # interleave by first use across both hwdge trigger queues:
            # sync feeds pair 0 (x b0 + rt h0), scalar feeds the rest
            for cc in range(2):
                nc.sync.dma_start(xb_sb[:, 0, cc], x8r[0, :, cc])
            load_rt(1, nc.scalar)
            nc.scalar.dma_start(xb_sb[:, 1], x8r[1])
            load_rt(0, nc.sync)
            for b in range(2, B):
                nc.scalar.dma_start(xb_sb[:, b], x8r[b])

            # ---- per-pair stage emitters ----
            def emit_qkv_mm(b, h, idx, nn):
                ps_qkv = pspool.tile(
                    [128, 2, 4, 128], F32, tag="big", bufs=1, name="ps_qkv"
                )
                for cc in range(2):
                    nc.tensor.matmul(
                        ps_qkv[0:96, nn],
                        lhsT=wq_sb[:, cc, :, h, :],
                        rhs=xb_sb[:, b, cc, :, 512 * nn : 512 * nn + 512],
                        start=(cc == 0),
                        stop=(cc == 1),
                        perf_mode=mybir.MatmulPerfMode.DoubleRow,
                    )
                return ps_qkv

            def emit_qkv_evict(h, ps_qkv, qkv_hold, nn):
                # bias-add eviction on ACT (keeps DVE free; bias is a
                # per-partition scalar so ACT can fuse it)
                nc.scalar.activation(
                    qkv_hold[:, 4 * nn : 4 * nn + 4, :],
                    ps_qkv[0:96, nn],
                    AF.Identity,
                    bias=bq_sb[:, h : h + 1],
                )

            def emit_repl(qkv_hold):
                # replicate q to row groups 1-3; k chunk-blocks to groups
                # 0-1 (jc 0-3) and 2-3 (jc 4-7); transpose v in one DMA
                q4 = work.tile([128, 8, 128], F16, tag="q4")
                kk = work.tile([128, 4, 128], F16, tag="kk")
                vtc = work.tile([128, 8, 32], F16, tag="vtc")
                vt1 = work.tile([128, 8, 34], F16, tag="vt1")
                for g in range(1, 4):
                    nc.sync.dma_start(q4[32 * g : 32 * g + 32], qkv_hold[0:32])
                for g in range(4):
                    nc.gpsimd.dma_start(
                        kk[32 * g : 32 * g + 32],
                        qkv_hold[32:64, 4 * (g // 2) : 4 * (g // 2) + 4, :],
                    )
                # xbar transpose needs a fully contiguous destination; the
                # strided [.., 0:32] view of vt1 is re-packed by a cheap 4x
                # DVE copy that also leaves room for the ones column
                nc.sync.dma_start_transpose(vtc[:], qkv_hold[64:96])
                nc.vector.tensor_copy(vt1[:, :, 0:32], vtc[:])
                nc.vector.memset(vt1[:, :, 32:33], 1.0)
                return q4, kk, vt1

            def emit_mid(b, h, qkv_hold, q4, kk, vt1, deferred, fin_late, inject):
                p2 = work.tile([128, 2, 4, 2, 512], F16, tag="p2")
                att_ps = pspool.tile([128, 512], F32, tag="att", bufs=2)

                def attv(r):
                    # the two i-half quadrants run concurrently as column
                    # tiles (0,0)/(0,64) - each M=33 fits a 64-col tile
                    for cg in range(4):
                        c, nn = cg // 2, cg % 2
                        jc = 4 * c + r
                        nc.tensor.matmul(
                            att_ps[64 * nn : 64 * nn + 33, :],
                            lhsT=vt1[:, jc, 0:33],
                            rhs=p2[:, c, r, nn, :],
                            start=(jc == 0),
                            stop=(jc == 7),
                            tile_position=(0, 64 * nn),
                        )

                def chunk(sc_ps, r, c):
                    jc = 4 * c + r
                    if jc in SCH_JC:
                        # fused exp-approx: int16(S*C1/256 + R'') bits = fp16
                        nc.vector.scalar_tensor_tensor(
                            out=p2[:, c, r].bitcast(I16),
                            in0=sc_ps[:],
                            scalar=C1 * SEXP,
                            in1=expRT[h][:, c, r],
                            op0=OP.mult,
                            op1=OP.add,
                        )
                    else:
                        nc.scalar.activation(
                            p2[:, c, r], sc_ps[:], AF.Exp, scale=SEXP
                        )
                        eng = nc.gpsimd if jc in GPS_JC else nc.vector
                        eng.tensor_tensor(
                            p2[:, c, r], p2[:, c, r], expRT[h][:, c, r], OP.mult
                        )

                for r in range(4):
                    for c in range(2):
                        # half-round: one jc chunk, both i-halves, on its own
                        # 2-bank psum (bufs=2 gives 2 half-rounds of slack so
                        # score matmuls never wait on the softmax consumers)
                        sc_ps = pspool.tile(
                            [128, 2, 512], F32, tag="sc", bufs=2, name="sc_ps"
                        )
                        for nn in range(2):
                            g = 2 * c + nn
                            nc.tensor.matmul(
                                sc_ps[:, nn, :],
                                lhsT=kk[32 * g : 32 * g + 32, r, :],
                                rhs=(qkv_hold if g == 0 else q4)[
                                    32 * g : 32 * g + 32, 4 * nn : 4 * nn + 4, :
                                ],
                                start=True,
                                stop=True,
                                tile_position=(32 * g, 0),
                            )
                        chunk(sc_ps, r, c)
                    if r >= 2:
                        attv(r - 2)
                    if r in (1, 2) and inject:
                        inject.pop(0)()
                    if r >= 2 and deferred:
                        deferred.pop(0)()
                attv(2)
                attv(3)
                if deferred:
                    deferred.pop(0)()
                return att_ps

            def emit_fin_early(att_ps):
                # denominators: rows 32/96 of att_ps -> sbuf (DVE)
                cs = work.tile([128, 512], F16, tag="cs")
                for nn in range(2):
                    rr = 64 * nn + 32
                    nc.vector.tensor_copy(cs[rr : rr + 1, :], att_ps[rr : rr + 1, :])
                return cs

            def emit_fin_late(att_ps, cs, dst, dst_half):
                bc_ps = pspool.tile([128, 512], F32, tag="att", bufs=2, name="bc_ps")
                for nn in range(2):
                    rr = 64 * nn + 32
                    nc.tensor.matmul(
                        bc_ps[64 * nn : 64 * nn + 32, :],
                        lhsT=ones_bc[rr : rr + 1, 0:32],
                        rhs=cs[rr : rr + 1, :],
                        start=True,
                        stop=True,
                        tile_position=(rr, 64 * nn),
                    )
                rcp = work.tile([128, 512], F32, tag="rcp")
                nc.vector.reciprocal_approx_fast(rcp[:], bc_ps[:])
                nc.vector.tensor_tensor(dst[:], att_ps[:], rcp[:], OP.mult)
                if dst_half is not None:
                    # h1: shift quadrants down 32 partitions into the proj rhs
                    nc.sync.dma_start(dst_half[32:64], dst[0:32])
                    nc.sync.dma_start(dst_half[96:128], dst[64:96])

            def make_proj(b, proj_rhs):
                # 4 deferred chunks; each: two concurrent 64x128 row-tiled
                # matmuls (K=64 contracts both heads), one 2-bank eviction,
                # output DMA
                out_sb = work.tile([128, 4, 2, 512], F16, tag="out_sb")
                outr = outp[b].rearrange("(oc p) (u n) -> p oc u n", p=128, u=2)

                def chunk(oc):
                    def run():
                        # the last batch's chunks run in the drain when the
                        # score psum is idle - rotate through its buffers so
                        # consecutive chunks never wait on each other's evict
                        if b == B - 1:
                            pj = pspool.tile(
                                [128, 2, 512], F32, tag="sc", bufs=2, name="sc_ps"
                            )
                        else:
                            pj = pspool.tile(
                                [128, 2, 512], F32, tag="big", bufs=1, name="pj"
                            )
                        for nn in range(2):
                            nc.tensor.matmul(
                                pj[:, nn, :],
                                lhsT=wp_sb[64 * nn : 64 * nn + 64, oc, :],
                                rhs=proj_rhs[64 * nn : 64 * nn + 64, :],
                                start=True,
                                stop=True,
                                tile_position=(64 * nn, 0),
                            )
                        nc.scalar.activation(out_sb[:, oc], pj[:], AF.Identity)
                        nc.sync.dma_start(outr[:, oc], out_sb[:, oc])

                    return run

                return [chunk(oc) for oc in range(4)]

            # ---- main loop, software-pipelined across pairs ----
            pairs = [(b, h) for b in range(B) for h in range(HPC)]
            qkv_tiles = {}
            proj_tiles = {}
            deferred = []

            def make_qkv_inject(idx):
                b, h = pairs[idx]

                def half0():
                    qkv_hold = work.tile([96, 8, 128], F16, tag="qkv_hold")
                    ps_qkv = emit_qkv_mm(b, h, idx, 0)
                    emit_qkv_evict(h, ps_qkv, qkv_hold, 0)
                    qkv_tiles[idx] = [qkv_hold, ps_qkv, None, None, None]

                def half1():
                    st = qkv_tiles[idx]
                    qkv_hold, ps_qkv = st[0], st[1]
                    for cc in range(2):
                        nc.tensor.matmul(
                            ps_qkv[0:96, 1],
                            lhsT=wq_sb[:, cc, :, h, :],
                            rhs=xb_sb[:, b, cc, :, 512:1024],
                            start=(cc == 0),
                            stop=(cc == 1),
                            perf_mode=mybir.MatmulPerfMode.DoubleRow,
                        )
                    emit_qkv_evict(h, ps_qkv, qkv_hold, 1)
                    st[2], st[3], st[4] = emit_repl(qkv_hold)

                return [half0, half1]

            def make_fin_late(pb, ph, patt, cs):
                def run():
                    if ph == 0:
                        proj_rhs = work.tile([128, 512], F16, tag="proj_rhs")
                        proj_tiles[pb] = proj_rhs
                        emit_fin_late(patt, cs, proj_rhs, None)
                    else:
                        proj_rhs = proj_tiles.pop(pb)
                        att_h1 = work.tile([128, 512], F16, tag="att_h1")
                        emit_fin_late(patt, cs, att_h1, proj_rhs)
                        deferred.extend(make_proj(pb, proj_rhs))

                return run

            for fn in make_qkv_inject(0):
                fn()
            for idx, (b, h) in enumerate(pairs):
                qkv_hold, _ps, q4, kk, vt1 = qkv_tiles.pop(idx)
                inject = make_qkv_inject(idx + 1) if idx + 1 < len(pairs) else []
                att_ps = emit_mid(
                    b, h, qkv_hold, q4, kk, vt1, deferred, None, inject
                )
                # fin chain inline at the pair tail: the DVE/PE ops land
                # ahead of the next pair's queue work, so proj and the next
                # pair's scores never wait on a deep queue
                cs = emit_fin_early(att_ps)
                make_fin_late(b, h, att_ps, cs)()
            for fn in deferred:
                fn()

    nc.compile()
    return nc


def _get_nc():
    global _BUILT
    if _BUILT is None:
        _BUILT = build_nc()
    return _BUILT


def _prep_inputs(x, w_qkv, b_qkv, w_proj, b_proj, shared_rel_pos):
    """Host-side sharding/layout prep. Returns per-core input maps."""
    scale = np.float32(DH**-0.5)
    x8 = np.ascontiguousarray(x.reshape(B, C, N)).astype(ml_dtypes.float8_e4m3)

    wq = w_qkv.reshape(HEADS, 96, C).astype(np.float32) * WSCALE
    wq[:, 0:32, :] *= scale  # fold attention scale into q
    bq = b_qkv.reshape(HEADS, 96).astype(np.float32) * WSCALE
    bq[:, 0:32] *= scale

    in_maps = []
    for g in range(NCORES):
        hh = [HPC * g + h for h in range(HPC)]
        wqkvT = np.ascontiguousarray(
            wq[hh].transpose(2, 0, 1).astype(ml_dtypes.float8_e4m3)
        )  # [C, HPC, 96]
        bqkv = np.ascontiguousarray(bq[hh].T)  # [96, HPC]
        # proj weight columns (un-scaled by 1/WSCALE): [64 (h d), 4 oc, 128]
        # stacked twice along partitions for the two 64x128 row tiles
        wp = w_proj[:, 64 * g : 64 * (g + 1)].astype(np.float32) / WSCALE
        wp64 = wp.T.reshape(64, 4, 128)
        wp64T = np.ascontiguousarray(
            np.concatenate([wp64, wp64], axis=0).astype(np.float16)
        )
        # rel-pos: exp(R^T) for ACT-exp chunks, Schraudolph R'' for SCH_JC
        rT = shared_rel_pos[0, hh].transpose(0, 2, 1).astype(np.float32)  # [h, j, i]
        rt = np.exp(rT)
        for jc in SCH_JC:
            sl = slice(128 * jc, 128 * (jc + 1))
            rt[:, sl, :] = rT[:, sl, :] * C1 + C2
        rt = np.ascontiguousarray(rt).astype(np.float16)
        in_maps.append(
            {"x8": x8, "wqkvT": wqkvT, "bqkv": bqkv, "wp64T": wp64T, "rt": rt}
        )
    return in_maps


def kernel(x, w_qkv, b_qkv, w_proj, b_proj, shared_rel_pos, _trace=False):
    nc = _get_nc()
    in_maps = _prep_inputs(x, w_qkv, b_qkv, w_proj, b_proj, shared_rel_pos)
    res = run_bass_kernel_spmd(nc, in_maps, list(range(NCORES)), trace=_trace)
    kernel.last_result = res
    out = np.zeros((B, C, N), np.float32)
    for g in range(NCORES):
        out += res.results[g]["outp"].astype(np.float32)
    out += b_proj.astype(np.float32)[None, :, None]
    return out.reshape(B, C, 32, 32).astype(np.float32)


# revision 34
# speedup vs baseline: 1.0558x; 1.0376x over previous
"""Attention2d Trainium2 kernel.

Sharding: 16 heads / 8 cores = 2 heads per core, data-parallel over all 4
batches on every core (head sharding minimizes rel_pos traffic: each core
reads only its 2 heads' [N, N] slices). The output projection contracts over
all heads' channels, so each core emits a partial projection output over its
64 channels; the host sums the 8 partials and adds b_proj.

Device pipeline per (batch, head) pair:
  qkv     = wqkv^T @ x_b           (PE; x and wqkv in fp8 with a x16 weight
                                    scale folded host-side; ACT bias-evict)
  v^T     single DMA-transpose [32,1024] -> [128,8,32] (no engine time)
  q, k    replicated across 32-partition row groups by SBUF->SBUF DMA so the
          K=32 score matmuls pack 4-wide via tile_position row tiling
  scores  4 rounds; round r = chunks jc=r (c0) and jc=r+4 (c1), each a
          [128, 1024] half-round -> one 4-bank psum tile; the softmax reads
          are split per half-round so the next round's matmuls only wait on
          their own bank pair (WAR at half-round granularity)
  p       per chunk, one of: ACT exp(scale=1/256) then *exp(R^T) on DVE
          (fp16 2x) or GpSimd (2 chunks, consumed 2 rounds later); or a
          fused Schraudolph exp-approx on DVE (scalar_tensor_tensor:
          int16(S*c1/256 + R''), bitcast fp16; R'' host-precomputed) for
          chunks jc in {3, 6, 7} - removes both the ACT op and the multiply
  att     [vt|1]^T @ p accumulated across rounds at lag 2 (PE 128x128 mode;
          i halves in PE output quadrants 0/64; ones column makes rows
          32/96 the denominators)
  rcp     denominator rows -> DVE copy -> PE ones-broadcast -> DVE recip
  att_sb  = att * rcp on DVE; h0 lands directly in the proj rhs tile, h1 is
          DMA-shifted so rows hold [h0|h1] x [i-half0|i-half1]
  out_b  += wp64^T @ att_sb        (PE 64x128 mode, K=64 contracts both
                                    heads, 2 concurrent row tiles; one
                                    2-bank eviction per output chunk)
"""

import sys

sys.path.insert(0, "/opt/trn_rl_repo")

import numpy as np
import ml_dtypes

import concourse.bass as bass
import concourse.tile as tile
from concourse import mybir, bacc
from concourse.bass_utils import run_bass_kernel_spmd

B, C = 4, 512
N = 1024  # 32*32 pixels
HEADS, DH = 16, 32
NCORES = 8
HPC = HEADS // NCORES  # heads per core
F16 = mybir.dt.float16
F32 = mybir.dt.float32
F8 = mybir.dt.float8e4
I16 = mybir.dt.int16
AF = mybir.ActivationFunctionType
OP = mybir.AluOpType

WSCALE = 16.0  # fp8 weight scale (q,k,v each x16 -> scores x256)
SEXP = 1.0 / (WSCALE * WSCALE)
C1 = 1477.3197  # 2^10 / ln(2)
C2 = 15315.27  # 15 * 1024 - 44.73 (L-inf centered Schraudolph)
SCH_JC = (0, 1, 4)  # chunks using the fused DVE exp-approx (early rounds so
# the pair boundary never waits on a deep DVE queue)
GPS_JC = (2,)  # chunks whose rel-pos multiply runs on GpSimd (jc2 is
# consumed in the pair tail, so the slow GpSimd op has slack)

_BUILT = None


def build_nc():
    nc = bacc.Bacc("TRN2", target_bir_lowering=False, debug=False, num_devices=NCORES)
    x8 = nc.declare_dram_parameter("x8", [B, C, N], F8, isOutput=False)
    wqkvT = nc.declare_dram_parameter("wqkvT", [C, HPC, 96], F8, isOutput=False)
    bqkv = nc.declare_dram_parameter("bqkv", [96, HPC], F32, isOutput=False)
    wp64T = nc.declare_dram_parameter("wp64T", [128, 4, 128], F16, isOutput=False)
    rt = nc.declare_dram_parameter("rt", [HPC, N, N], F16, isOutput=False)
    outp = nc.declare_dram_parameter("outp", [B, C, N], F16, isOutput=True)

    with tile.TileContext(nc) as tc:
        with (
            tc.tile_pool(name="singles", bufs=1) as singles,
            tc.tile_pool(name="work", bufs=2) as work,
            tc.tile_pool(name="ps", bufs=1, space="PSUM") as pspool,
        ):
            # ---- preamble: constants + resident tensors ----
            ones_bc = singles.tile([128, 32], F16)
            nc.vector.memset(ones_bc[:], 1.0)

            # DoubleRow channel pairing: c = cc*256 + ki*2 + ko on both the
            # weight and activation side (any consistent bijection works)
            wq_sb = singles.tile([128, 2, 2, HPC, 96], F8)
            nc.sync.dma_start(
                wq_sb[:], wqkvT.rearrange("(cc p ko) h m -> p cc ko h m", p=128, ko=2)
            )
            bq_sb = singles.tile([96, HPC], F32)
            nc.sync.dma_start(bq_sb[:], bqkv[:])
            wp_sb = singles.tile([128, 4, 128], F16)
            nc.sync.dma_start(wp_sb[:], wp64T[:])

            # input DMAs ordered by first use so pair 0 can start early
            xb_sb = singles.tile([128, B, 2, 2, N], F8)
            expRT = [
                singles.tile(
                    [128, 2, 4, 2, 512], F16, tag=f"expRT{h}", name=f"expRT{h}"
                )
                for h in range(HPC)
            ]
            x8r = x8.rearrange("b (cc p ko) n -> b p cc ko n", p=128, ko=2)
            rtr = rt.rearrange("h (jc p) (u n) -> h p jc u n", p=128, u=2)

            def load_rt(h, eng):
                for r in range(4):
                    for c in range(2):
                        eng.dma_start(expRT[h][:, c, r], rtr[h, :, 4 * c + r])

            # interleave by first use across both hwdge trigger queues:
            # sync feeds pair 0 (x b0 + rt h0), scalar feeds the rest
            for cc in range(2):
                nc.sync.dma_start(xb_sb[:, 0, cc], x8r[0, :, cc])
            load_rt(1, nc.scalar)
            nc.scalar.dma_start(xb_sb[:, 1], x8r[1])
            load_rt(0, nc.sync)
            for b in range(2, B):
                nc.scalar.dma_start(xb_sb[:, b], x8r[b])

            # ---- per-pair stage emitters ----
            def emit_qkv_mm(b, h, idx, nn):
                ps_qkv = pspool.tile(
                    [128, 2, 4, 128], F32, tag="big", bufs=1, name="ps_qkv"
                )
                for cc in range(2):
                    nc.tensor.matmul(
                        ps_qkv[0:96, nn],
                        lhsT=wq_sb[:, cc, :, h, :],
                        rhs=xb_sb[:, b, cc, :, 512 * nn : 512 * nn + 512],
                        start=(cc == 0),
                        stop=(cc == 1),
                        perf_mode=mybir.MatmulPerfMode.DoubleRow,
                    )
                return ps_qkv

            def emit_qkv_evict(h, ps_qkv, qkv_hold, nn):
                # bias-add eviction on ACT (keeps DVE free; bias is a
                # per-partition scalar so ACT can fuse it)
                nc.scalar.activation(
                    qkv_hold[:, 4 * nn : 4 * nn + 4, :],
                    ps_qkv[0:96, nn],
                    AF.Identity,
                    bias=bq_sb[:, h : h + 1],
                )

            def emit_repl(qkv_hold):
                # replicate q to row groups 1-3; k chunk-blocks to groups
                # 0-1 (jc 0-3) and 2-3 (jc 4-7); transpose v in one DMA
                q4 = work.tile([128, 8, 128], F16, tag="q4")
                kk = work.tile([128, 4, 128], F16, tag="kk")
                vtc = work.tile([128, 8, 32], F16, tag="vtc")
                vt1 = work.tile([128, 8, 34], F16, tag="vt1")
                for g in range(1, 4):
                    nc.sync.dma_start(q4[32 * g : 32 * g + 32], qkv_hold[0:32])
                for g in range(4):
                    nc.gpsimd.dma_start(
                        kk[32 * g : 32 * g + 32],
                        qkv_hold[32:64, 4 * (g // 2) : 4 * (g // 2) + 4, :],
                    )
                # xbar transpose needs a fully contiguous destination; the
                # strided [.., 0:32] view of vt1 is re-packed by a cheap 4x
                # DVE copy that also leaves room for the ones column
                nc.sync.dma_start_transpose(vtc[:], qkv_hold[64:96])
                nc.vector.tensor_copy(vt1[:, :, 0:32], vtc[:])
                nc.vector.memset(vt1[:, :, 32:33], 1.0)
                return q4, kk, vt1

            def emit_mid(b, h, qkv_hold, q4, kk, vt1, deferred, fin_late, inject):
                p2 = work.tile([128, 2, 4, 2, 512], F16, tag="p2")
                att_ps = pspool.tile([128, 512], F32, tag="att", bufs=2)

                def attv(r):
                    # the two i-half quadrants run concurrently as column
                    # tiles (0,0)/(0,64) - each M=33 fits a 64-col tile
                    for cg in range(4):
                        c, nn = cg // 2, cg % 2
                        jc = 4 * c + r
                        nc.tensor.matmul(
                            att_ps[64 * nn : 64 * nn + 33, :],
                            lhsT=vt1[:, jc, 0:33],
                            rhs=p2[:, c, r, nn, :],
                            start=(jc == 0),
                            stop=(jc == 7),
                            tile_position=(0, 64 * nn),
                        )

                def chunk(sc_ps, r, c):
                    jc = 4 * c + r
                    if jc in SCH_JC:
                        # fused exp-approx: int16(S*C1/256 + R'') bits = fp16
                        nc.vector.scalar_tensor_tensor(
                            out=p2[:, c, r].bitcast(I16),
                            in0=sc_ps[:],
                            scalar=C1 * SEXP,
                            in1=expRT[h][:, c, r],
                            op0=OP.mult,
                            op1=OP.add,
                        )
                    else:
                        nc.scalar.activation(
                            p2[:, c, r], sc_ps[:], AF.Exp, scale=SEXP
                        )
                        eng = nc.gpsimd if jc in GPS_JC else nc.vector
                        eng.tensor_tensor(
                            p2[:, c, r], p2[:, c, r], expRT[h][:, c, r], OP.mult
                        )

                for r in range(4):
                    for c in range(2):
                        # half-round: one jc chunk, both i-halves, on its own
                        # 2-bank psum (bufs=2 gives 2 half-rounds of slack so
                        # score matmuls never wait on the softmax consumers)
                        sc_ps = pspool.tile(
                            [128, 2, 512], F32, tag="sc", bufs=2, name="sc_ps"
                        )
                        for nn in range(2):
                            g = 2 * c + nn
                            nc.tensor.matmul(
                                sc_ps[:, nn, :],
                                lhsT=kk[32 * g : 32 * g + 32, r, :],
                                rhs=(qkv_hold if g == 0 else q4)[
                                    32 * g : 32 * g + 32, 4 * nn : 4 * nn + 4, :
                                ],
                                start=True,
                                stop=True,
                                tile_position=(32 * g, 0),
                            )
                        chunk(sc_ps, r, c)
                    if r >= 2:
                        attv(r - 2)
                    if r in (1, 2) and inject:
                        inject.pop(0)()
                    if r >= 2 and deferred:
                        deferred.pop(0)()
                attv(2)
                attv(3)
                if deferred:
                    deferred.pop(0)()
                return att_ps

            def emit_fin_early(att_ps):
                # denominators: rows 32/96 of att_ps -> sbuf (DVE)
                cs = work.tile([128, 512], F16, tag="cs")
                for nn in range(2):
                    rr = 64 * nn + 32
                    nc.vector.tensor_copy(cs[rr : rr + 1, :], att_ps[rr : rr + 1, :])
                return cs

            def emit_fin_late(att_ps, cs, dst, dst_half):
                bc_ps = pspool.tile([128, 512], F32, tag="att", bufs=2, name="bc_ps")
                for nn in range(2):
                    rr = 64 * nn + 32
                    nc.tensor.matmul(
                        bc_ps[64 * nn : 64 * nn + 32, :],
                        lhsT=ones_bc[rr : rr + 1, 0:32],
                        rhs=cs[rr : rr + 1, :],
                        start=True,
                        stop=True,
                        tile_position=(rr, 64 * nn),
                    )
                rcp = work.tile([128, 512], F32, tag="rcp")
                nc.vector.reciprocal_approx_fast(rcp[:], bc_ps[:])
                nc.vector.tensor_tensor(dst[:], att_ps[:], rcp[:], OP.mult)
                if dst_half is not None:
                    # h1: shift quadrants down 32 partitions into the proj rhs
                    nc.sync.dma_start(dst_half[32:64], dst[0:32])
                    nc.sync.dma_start(dst_half[96:128], dst[64:96])

            def make_proj(b, proj_rhs):
                # 4 deferred chunks; each: two concurrent 64x128 row-tiled
                # matmuls (K=64 contracts both heads), one 2-bank eviction,
                # output DMA
                out_sb = work.tile([128, 4, 2, 512], F16, tag="out_sb")
                outr = outp[b].rearrange("(oc p) (u n) -> p oc u n", p=128, u=2)

                def chunk(oc):
                    def run():
                        # the last batch's chunks run in the drain when the
                        # score psum is idle - rotate through its buffers so
                        # consecutive chunks never wait on each other's evict
                        if b == B - 1:
                            pj = pspool.tile(
                                [128, 2, 512], F32, tag="sc", bufs=2, name="sc_ps"
                            )
                        else:
                            pj = pspool.tile(
                                [128, 2, 512], F32, tag="big", bufs=1, name="pj"
                            )
                        for nn in range(2):
                            nc.tensor.matmul(
                                pj[:, nn, :],
                                lhsT=wp_sb[64 * nn : 64 * nn + 64, oc, :],
                                rhs=proj_rhs[64 * nn : 64 * nn + 64, :],
                                start=True,
                                stop=True,
                                tile_position=(64 * nn, 0),
                            )
                        if b == B - 1 and oc % 2 == 1:
                            # drain phase: alternate evict engines so the
                            # final chunks pace on two queues, not one
                            nc.vector.tensor_copy(out_sb[:, oc], pj[:])
                        else:
                            nc.scalar.activation(out_sb[:, oc], pj[:], AF.Identity)
                        nc.sync.dma_start(outr[:, oc], out_sb[:, oc])

                    return run

                return [chunk(oc) for oc in range(4)]

            # ---- main loop, software-pipelined across pairs ----
            pairs = [(b, h) for b in range(B) for h in range(HPC)]
            qkv_tiles = {}
            proj_tiles = {}
            deferred = []

            def make_qkv_inject(idx):
                b, h = pairs[idx]

                def half0():
                    qkv_hold = work.tile([96, 8, 128], F16, tag="qkv_hold")
                    ps_qkv = emit_qkv_mm(b, h, idx, 0)
                    emit_qkv_evict(h, ps_qkv, qkv_hold, 0)
                    qkv_tiles[idx] = [qkv_hold, ps_qkv, None, None, None]

                def half1():
                    st = qkv_tiles[idx]
                    qkv_hold, ps_qkv = st[0], st[1]
                    for cc in range(2):
                        nc.tensor.matmul(
                            ps_qkv[0:96, 1],
                            lhsT=wq_sb[:, cc, :, h, :],
                            rhs=xb_sb[:, b, cc, :, 512:1024],
                            start=(cc == 0),
                            stop=(cc == 1),
                            perf_mode=mybir.MatmulPerfMode.DoubleRow,
                        )
                    emit_qkv_evict(h, ps_qkv, qkv_hold, 1)
                    st[2], st[3], st[4] = emit_repl(qkv_hold)

                return [half0, half1]

            def make_fin_late(pb, ph, patt, cs):
                def run():
                    if ph == 0:
                        proj_rhs = work.tile([128, 512], F16, tag="proj_rhs")
                        proj_tiles[pb] = proj_rhs
                        emit_fin_late(patt, cs, proj_rhs, None)
                    else:
                        proj_rhs = proj_tiles.pop(pb)
                        att_h1 = work.tile([128, 512], F16, tag="att_h1")
                        emit_fin_late(patt, cs, att_h1, proj_rhs)
                        deferred.extend(make_proj(pb, proj_rhs))

                return run

            for fn in make_qkv_inject(0):
                fn()
            for idx, (b, h) in enumerate(pairs):
                qkv_hold, _ps, q4, kk, vt1 = qkv_tiles.pop(idx)
                inject = make_qkv_inject(idx + 1) if idx + 1 < len(pairs) else []
                att_ps = emit_mid(
                    b, h, qkv_hold, q4, kk, vt1, deferred, None, inject
                )
                # fin chain inline at the pair tail: the DVE/PE ops land
                # ahead of the next pair's queue work, so proj and the next
                # pair's scores never wait on a deep queue
                cs = emit_fin_early(att_ps)
                make_fin_late(b, h, att_ps, cs)()
            for fn in deferred:
                fn()

    nc.compile()
    return nc


def _get_nc():
    global _BUILT
    if _BUILT is None:
        _BUILT = build_nc()
    return _BUILT


def _prep_inputs(x, w_qkv, b_qkv, w_proj, b_proj, shared_rel_pos):
    """Host-side sharding/layout prep. Returns per-core input maps."""
    scale = np.float32(DH**-0.5)
    x8 = np.ascontiguousarray(x.reshape(B, C, N)).astype(ml_dtypes.float8_e4m3)

    wq = w_qkv.reshape(HEADS, 96, C).astype(np.float32) * WSCALE
    wq[:, 0:32, :] *= scale  # fold attention scale into q
    bq = b_qkv.reshape(HEADS, 96).astype(np.float32) * WSCALE
    bq[:, 0:32] *= scale

    in_maps = []
    for g in range(NCORES):
        hh = [HPC * g + h for h in range(HPC)]
        wqkvT = np.ascontiguousarray(
            wq[hh].transpose(2, 0, 1).astype(ml_dtypes.float8_e4m3)
        )  # [C, HPC, 96]
        bqkv = np.ascontiguousarray(bq[hh].T)  # [96, HPC]
        # proj weight columns (un-scaled by 1/WSCALE): [64 (h d), 4 oc, 128]
        # stacked twice along partitions for the two 64x128 row tiles
        wp = w_proj[:, 64 * g : 64 * (g + 1)].astype(np.float32) / WSCALE
        wp64 = wp.T.reshape(64, 4, 128)
        wp64T = np.ascontiguousarray(
            np.concatenate([wp64, wp64], axis=0).astype(np.float16)
        )
        # rel-pos: exp(R^T) for ACT-exp chunks, Schraudolph R'' for SCH_JC
        rT = shared_rel_pos[0, hh].transpose(0, 2, 1).astype(np.float32)  # [h, j, i]
        rt = np.exp(rT)
        for jc in SCH_JC:
            sl = slice(128 * jc, 128 * (jc + 1))
            rt[:, sl, :] = rT[:, sl, :] * C1 + C2
        rt = np.ascontiguousarray(rt).astype(np.float16)
        in_maps.append(
            {"x8": x8, "wqkvT": wqkvT, "bqkv": bqkv, "wp64T": wp64T, "rt": rt}
        )
    return in_maps


def kernel(x, w_qkv, b_qkv, w_proj, b_proj, shared_rel_pos, _trace=False):
    nc = _get_nc()
    in_maps = _prep_inputs(x, w_qkv, b_qkv, w_proj, b_proj, shared_rel_pos)
    res = run_bass_kernel_spmd(nc, in_maps, list(range(NCORES)), trace=_trace)
    kernel.last_result = res
    out = np.zeros((B, C, N), np.float32)
    for g in range(NCORES):
        out += res.results[g]["outp"].astype(np.float32)
    out += b_proj.astype(np.float32)[None, :, None]
    return out.reshape(B, C, 32, 32).astype(np.float32)
